# revision 24
# baseline (speedup 1.0000x reference)
"""CFConvS2V Trainium2 kernel (8-core data-parallel over batch).

reference computation:
    h = silu(layernorm(s @ W1.T + b1))               # (B, N, H)
    v[b,i,c,d] = sum_j mask[b,i,j] * ev[b,i,j,c] * h[b,j,d]   # (B, N, 3, H)

Sharding: data-parallel over B across 8 cores (4 batches each); the pairwise
tensors and the j-reduction stay local per core.

The problem is HBM-bound (ev alone is 12 MiB/core in f32), so everything
rides in bf16: ev/mask/s stream in as bf16 (mask is 0/1 so bf16 is exact),
v streams out as bf16 and is upcast on host. That halves HBM traffic vs the
f32 baseline. ev is pre-transposed on host to [j_local, c, (jc,it,i)] so the
contraction over j needs NO on-device transposes: the masked ev chunks feed
the PE directly as the moving operand with h[jc] stationary.

Per-core plan (B_loc=4, N=512, H=128, C=3), per batch:
  h-phase: rank-1 matmul seeds PSUM with b1 (ones^T @ b1rep), 4 bf16 matmuls
  accumulate s @ W1.T on top; LayerNorm stats via bn_stats/bn_aggr reading
  PSUM; rstd comes from a fast-inverse-sqrt seed + 1 Newton step on the DVE
  (batched [p,4], contiguous operands) so ACT only ever runs Silu/Copy --
  both in the silu_and_others table set, i.e. zero ~1.3us ACT_TABLE_LOADs in
  steady state (the naive Sqrt<->Silu alternation costs ~33us/iter in table
  swaps). ACT then computes h = Silu(psum*rstd - mu*rstd) per tile straight
  from PSUM into bf16 SBUF (fused: scale/bias are per-partition APs).
  i-phase: one 1.5 MiB DMA brings evT[b], one 0.5 MiB DMA brings maskT[b];
  a single DVE multiply applies the mask (broadcast over c on the outer free
  dim, operands contiguous so the 16-bit packed 2x mode engages); 12 bf16
  matmuls (h[jc] stationary, mev[c] 512-wide moving = one full PSUM bank of
  f32 out, the ISA max; jc-inner keeps 4 back-to-back MMs on the same bank)
  accumulate v[d, c, (it,i)] into 3 PSUM banks; ACT evicts to bf16 and the
  store rides the ACT HWDGE ring so it can't block the SP-ring loads. Host
  reorders [d, c, n] -> [n, c, d] and upcasts to f32.
"""

import sys

if "/opt/trn_rl_repo" not in sys.path:
    sys.path.insert(0, "/opt/trn_rl_repo")

from contextlib import ExitStack

import numpy as np
import ml_dtypes

import concourse.bass as bass
import concourse.mybir as mybir
from concourse.tile import TileContext

B, N, H, C = 32, 512, 128, 3
NCORES = 8
BL = B // NCORES      # batches per core
P = 128
NT = N // P           # i-tiles per batch
JC = N // P           # j-chunks
LN_EPS = 1e-5
F32 = mybir.dt.float32
BF16 = mybir.dt.bfloat16
AF = mybir.ActivationFunctionType
BF16NP = ml_dtypes.bfloat16

JNP = JC * NT * P     # flattened (jc, it, i) extent = 2048


def _split_multi_waits(nc):
    """The walrus build in this container only accepts one sync-wait per
    instruction; hoist extra waits onto single-wait NOPs in front."""
    ctr = 0
    for f in nc.m.functions:
        for bb in f.blocks:
            insts = bb.instructions
            i = 0
            while i < len(insts):
                inst = insts[i]
                si = inst.sync_info
                if si is not None and len(si.on_wait) > 1:
                    waits = list(si.on_wait)
                    for w in waits[:-1]:
                        ctr += 1
                        nop = mybir.InstNoOp(
                            name=f"splitwait-{ctr}",
                            engine=inst.engine,
                            sync_info=mybir.SyncInfo(on_wait=[w], on_update=[]),
                            bass_nofuse=True,
                        )
                        nc.register_instruction(nop, overwrite=True)
                        insts.insert(i, nop)
                        i += 1
                    inst.sync_info = mybir.SyncInfo(
                        on_wait=[waits[-1]], on_update=list(si.on_update)
                    )
                i += 1


def build(reps=1):
    nc = bass.Bass("TRN2", target_bir_lowering=False, debug=False, num_devices=NCORES)
    evT = nc.dram_tensor("evT", [BL, P, C * JNP], BF16, kind="ExternalInput").ap()
    maskT = nc.dram_tensor("maskT", [BL, P, JNP], BF16, kind="ExternalInput").ap()
    sT = nc.dram_tensor("sT", [BL, H, N], BF16, kind="ExternalInput").ap()
    w1t = nc.dram_tensor("w1t", [H, H], BF16, kind="ExternalInput").ap()
    b1rep = nc.dram_tensor("b1rep", [1, NT * H], BF16, kind="ExternalInput").ap()
    # out column order: [d, c(3), it(4), i(128)]
    out = nc.dram_tensor("out", [BL, H, NT * C * P], BF16, kind="ExternalOutput").ap()

    with TileContext(nc) as tc, ExitStack() as ctx:
        const = ctx.enter_context(tc.tile_pool(name="const", bufs=1))
        p_ev = ctx.enter_context(tc.tile_pool(name="p_ev", bufs=2))
        p_mask = ctx.enter_context(tc.tile_pool(name="p_mask", bufs=2))
        p_mev = ctx.enter_context(tc.tile_pool(name="p_mev", bufs=2))
        p_vout = ctx.enter_context(tc.tile_pool(name="p_vout", bufs=2))
        p_sT = ctx.enter_context(tc.tile_pool(name="p_sT", bufs=2))
        p_h = ctx.enter_context(tc.tile_pool(name="p_h", bufs=2))
        p_stat = ctx.enter_context(tc.tile_pool(name="p_stat", bufs=4))
        ps_h = ctx.enter_context(tc.tile_pool(name="ps_h", bufs=2, space="PSUM"))
        ps_v = ctx.enter_context(tc.tile_pool(name="ps_v", bufs=2, space="PSUM"))

        w1t_sb = const.tile([H, H], BF16)
        nc.sync.dma_start(out=w1t_sb[:], in_=w1t[:])
        b1rep_sb = const.tile([1, NT * H], BF16)
        nc.sync.dma_start(out=b1rep_sb[:], in_=b1rep[:])
        ones_sb = const.tile([1, P], BF16)
        nc.vector.memset(ones_sb[:], 1.0)

        def body():
          for b in range(BL):
            # ---------- h phase: h = silu(LN(s @ W1.T + b1)) ----------
            sT_sb = p_sT.tile([H, N], BF16)
            nc.sync.dma_start(out=sT_sb[:], in_=sT[b])
            # mask rides the ACT HWDGE ring (nearly idle: only stores),
            # keeping the SP ring a pure sT+ev stream -> ev chunks land
            # ~1us earlier per batch and the first TT unblocks sooner
            mk_sb = p_mask.tile([P, JNP], BF16)
            nc.scalar.dma_start(out=mk_sb[:], in_=maskT[b])
            ev_cs = []
            for c in range(C):
                ev_c = p_ev.tile([P, JNP], BF16, tag=f"ev{c}")
                nc.sync.dma_start(
                    out=ev_c[:], in_=evT[b, :, c * JNP : (c + 1) * JNP]
                )
                ev_cs.append(ev_c)

            psum_h = ps_h.tile([P, NT * H], F32)
            # seed all of PSUM with b1 (rank-1: ones^T @ b1rep), then
            # accumulate the 4 n-tile matmuls on top
            nc.tensor.matmul(
                out=psum_h[:],
                lhsT=ones_sb[:],
                rhs=b1rep_sb[:],
                start=True,
                stop=False,
                skip_group_check=True,
            )
            for t in range(NT):
                # out[n_local, k] = sum_h sT[h, n] * W1T[h, k]
                nc.tensor.matmul(
                    out=psum_h[:, t * H : (t + 1) * H],
                    lhsT=sT_sb[:, t * P : (t + 1) * P],
                    rhs=w1t_sb[:],
                    start=False,
                    stop=True,
                    skip_group_check=True,
                )

            # LN stats straight off PSUM; rstd entirely on DVE (Newton with
            # fast-inverse-sqrt seed) so ACT only ever runs Silu/Copy — both
            # live in the silu_and_others table set, so NO ~1.3us
            # ACT_TABLE_LOADs in steady state (Sqrt lives in another set).
            mvall = p_stat.tile([P, NT, 2], F32, tag="mv")
            for t in range(NT):
                stats = p_stat.tile([P, 6], F32, tag="stats")
                nc.vector.bn_stats(
                    out=stats[:], in_=psum_h[:, t * H : (t + 1) * H]
                )
                nc.vector.bn_aggr(out=mvall[:, t, :], in_=stats[:])
            # rstd = 1/sqrt(var) via fast-inverse-sqrt seed + 1 Newton step
            # (~0.17% max err, far under the bf16 noise floor; eps dropped --
            # var ~ 1 for LN'd activations so var+1e-5 == var to 5 digits).
            # var is copied contiguous first: strided-bitcast inputs push the
            # DVE onto its slow-table ucode (~4x per-op cost).
            v4t = p_stat.tile([P, NT], F32, tag="v4")
            nc.vector.tensor_scalar(
                out=v4t[:], in0=mvall[:, :, 1], scalar1=0.0, scalar2=None,
                op0=mybir.AluOpType.add,
            )
            v4 = v4t[:]
            yi4 = p_stat.tile([P, NT], mybir.dt.int32, tag="yi4")
            nc.vector.tensor_scalar(
                out=yi4[:], in0=v4.bitcast(mybir.dt.int32), scalar1=1,
                scalar2=None, op0=mybir.AluOpType.logical_shift_right,
            )
            nc.vector.tensor_scalar(
                out=yi4[:], in0=yi4[:], scalar1=-1, scalar2=0x5F3759DF,
                op0=mybir.AluOpType.mult, op1=mybir.AluOpType.add,
            )
            rstd4 = yi4[:].bitcast(F32)
            t14 = p_stat.tile([P, NT], F32, tag="t14")
            nc.vector.tensor_mul(out=t14[:], in0=rstd4, in1=rstd4)
            nc.vector.tensor_mul(out=t14[:], in0=t14[:], in1=v4)
            nc.vector.tensor_scalar(
                out=t14[:], in0=t14[:], scalar1=-0.5, scalar2=1.5,
                op0=mybir.AluOpType.mult, op1=mybir.AluOpType.add,
            )
            nc.vector.tensor_mul(out=yi4[:].bitcast(F32), in0=rstd4, in1=t14[:])
            # nmr = -mu * rstd
            nmr4 = p_stat.tile([P, NT], F32, tag="nmr4")
            nc.vector.tensor_mul(out=nmr4[:], in0=mvall[:, :, 0], in1=rstd4)
            nc.vector.tensor_scalar(
                out=nmr4[:], in0=nmr4[:], scalar1=-1.0, scalar2=None,
                op0=mybir.AluOpType.mult,
            )
            h_sb = p_h.tile([P, NT, H], BF16)
            for t in range(NT):
                # h = Silu(x * rstd - mu * rstd) straight from PSUM -> bf16
                nc.scalar.activation(
                    out=h_sb[:, t, :],
                    in_=psum_h[:, t * H : (t + 1) * H],
                    func=AF.Silu,
                    bias=nmr4[:, t : t + 1],
                    scale=rstd4[:, t : t + 1],
                )

            # ---------- i phase ----------
            # per-c chains: mev_c = ev_c * mask (purely contiguous TT, no
            # broadcast AP), then 4 MMs consume it -- c-granular tiles let
            # the PE start on c=0 while c=1,2 are still loading/multiplying
            psum_v = ps_v.tile([P, C, NT * P], F32)
            for c in range(C):
                mev_c = p_mev.tile([P, JNP], BF16, tag=f"mev{c}")
                nc.vector.tensor_tensor(
                    out=mev_c[:],
                    in0=ev_cs[c][:],
                    in1=mk_sb[:],
                    op=mybir.AluOpType.mult,
                )
                for jc in range(JC):
                    nc.tensor.matmul(
                        out=psum_v[:, c, :],
                        lhsT=h_sb[:, jc, :],
                        rhs=mev_c[:, jc * NT * P : (jc + 1) * NT * P],
                        start=(jc == 0),
                        stop=(jc == JC - 1),
                        skip_group_check=True,
                    )

            vout = p_vout.tile([P, C, NT * P], BF16)
            nc.scalar.activation(out=vout[:], in_=psum_v[:], func=AF.Copy)
            # store on the ACT HWDGE ring so a compute-gated store can't
            # block the next batch's loads on the SP HWDGE FIFO
            nc.scalar.dma_start(
                out=out[b], in_=vout[:].rearrange("p c x -> p (c x)")
            )

        if reps == 1:
            body()
        else:
            with tc.For_i(0, reps, 1):
                body()

    _split_multi_waits(nc)
    return nc


_built_nc = None


def _get_nc():
    global _built_nc
    if _built_nc is None:
        _built_nc = build()
    return _built_nc


def shard_inputs(s, ev, mask, W1, b1):
    """Full inputs -> list of per-core input dicts (bf16, pre-transposed)."""
    s = np.asarray(s, dtype=np.float32)
    ev = np.asarray(ev, dtype=np.float32)
    mask = np.asarray(mask, dtype=np.float32)
    W1 = np.asarray(W1, dtype=np.float32)
    b1 = np.asarray(b1, dtype=np.float32)
    w1t = np.ascontiguousarray(W1.T).astype(BF16NP)
    b1rep = np.tile(b1, NT)[None, :].astype(BF16NP)
    in_maps = []
    for m in range(NCORES):
        bs = slice(m * BL, (m + 1) * BL)
        # ev[b, i, j, c] -> evT[b, j_local, c, jc, it, i_local]
        evm = ev[bs].reshape(BL, NT, P, JC, P, C)
        evm = evm.transpose(0, 4, 5, 3, 1, 2).reshape(BL, P, C * JNP)
        # mask[b, i, j, 1] -> maskT[b, j_local, jc, it, i_local]
        mkm = mask[bs].reshape(BL, NT, P, JC, P)
        mkm = mkm.transpose(0, 4, 3, 1, 2).reshape(BL, P, JNP)
        in_maps.append(
            {
                "evT": np.ascontiguousarray(evm).astype(BF16NP),
                "maskT": np.ascontiguousarray(mkm).astype(BF16NP),
                "sT": np.ascontiguousarray(s[bs].transpose(0, 2, 1)).astype(BF16NP),
                "w1t": w1t,
                "b1rep": b1rep,
            }
        )
    return in_maps


def unshard_output(per_core_outs):
    """list of per-core "out" arrays [BL, H, C*N] -> full (B, N, 3, H).

    Device column order is [d, c(3), n(512)]."""
    parts = []
    for o in per_core_outs:
        o = np.asarray(o, dtype=np.float32).reshape(BL, H, C, N)
        parts.append(np.ascontiguousarray(o.transpose(0, 3, 2, 1)))
    return np.concatenate(parts, axis=0)


_executor = None


def _get_executor():
    """Build the sharded PJRT executable once; reuse across kernel() calls."""
    global _executor
    if _executor is not None:
        return _executor
    import jax
    from jax.sharding import Mesh, PartitionSpec
    from jax.experimental.shard_map import shard_map

    from concourse import bass2jax

    bass2jax.install_neuronx_cc_hook()
    nc = _get_nc()
    partition_name = nc.partition_id_tensor.name if nc.partition_id_tensor else None
    in_names, out_names, out_avals, zero_outs = [], [], [], []
    for alloc in nc.m.functions[0].allocations:
        if not isinstance(alloc, mybir.MemoryLocationSet):
            continue
        name = alloc.memorylocations[0].name
        if alloc.kind == "ExternalInput":
            if name != partition_name:
                in_names.append(name)
        elif alloc.kind == "ExternalOutput":
            out_names.append(name)
            shape = tuple(alloc.tensor_shape)
            dtype = mybir.dt.np(alloc.dtype)
            out_avals.append(jax.core.ShapedArray(shape, dtype))
            zero_outs.append(np.zeros(shape, dtype))
    n_params = len(in_names)
    all_in_names = list(in_names) + list(out_names)
    if partition_name is not None:
        all_in_names.append(partition_name)

    def _body(*args):
        operands = list(args)
        if partition_name is not None:
            operands.append(bass2jax.partition_id_tensor())
        outs = bass2jax._bass_exec_p.bind(
            *operands,
            out_avals=tuple(out_avals),
            in_names=tuple(all_in_names),
            out_names=tuple(out_names),
            lowering_input_output_aliases=(),
            sim_require_finite=True,
            sim_require_nnan=True,
            nc=nc,
        )
        return tuple(outs)

    devices = jax.devices()[:NCORES]
    mesh = Mesh(np.asarray(devices), ("core",))
    donate = tuple(range(n_params, n_params + len(out_names)))
    fn = jax.jit(
        shard_map(
            _body,
            mesh=mesh,
            in_specs=(PartitionSpec("core"),) * (n_params + len(out_names)),
            out_specs=(PartitionSpec("core"),) * len(out_names),
            check_rep=False,
        ),
        donate_argnums=donate,
        keep_unused=True,
    )
    _executor = (fn, in_names, out_names, out_avals, zero_outs)
    return _executor


def kernel(s, ev, mask, W1, b1):
    fn, in_names, out_names, out_avals, zero_outs = _get_executor()
    in_maps = shard_inputs(s, ev, mask, W1, b1)
    concat_in = [
        np.concatenate([in_maps[c][nm] for c in range(NCORES)], axis=0)
        for nm in in_names
    ]
    concat_zeros = [
        np.zeros((NCORES * z.shape[0], *z.shape[1:]), z.dtype) for z in zero_outs
    ]
    out_arrs = fn(*concat_in, *concat_zeros)
    i = out_names.index("out")
    o = np.asarray(out_arrs[i]).reshape(NCORES, *out_avals[i].shape)
    return unshard_output([o[c] for c in range(NCORES)])
